# revision 1
# baseline (speedup 1.0000x reference)
"""Trainium2 Bass kernel for nn_Encoder_3075196584282 (sparse 1.5-entmax attention encoder).

Self-contained: kernel(**inputs) takes full f32 inputs, shards across 8 NeuronCores
(data-parallel: core = batch*2 + query_half; K/V computed per-core from its batch),
runs one SPMD Bass program, returns the full (4,1024,1024) f32 output.

Entmax-1.5 threshold per row is found with 4 Newton iterations from a
regression-based init (tau0 = c0 + c1*mean(z) + c2*std(z) - margin), using the
identities  sum_{z>t} (z-t) = sum max(z,t) - n*t  and, for z,t<=0,
sum_{z>t}(z-t)^2 = sum min(z^2,t^2) - 2t*sum max(z,t) + n*t^2  (support count
cancels), so each iteration is two tensor_scalar accumulate passes at 4x rate.
Attention output uses p = (u - tau)^2 with u = max(x^T, tau) in transposed
layout:  o = A2 - 2*tau*A1 + tau^2*A0  with A2 = u^2 @ V, A1 = u @ V, A0 = 1 @ V.
"""
import math
import numpy as np
from contextlib import ExitStack

import concourse.bass as bass
import concourse.bacc as bacc
import concourse.tile as tile
from concourse import mybir, bass_isa, library_config
from concourse.masks import make_identity

f32, f16 = mybir.dt.float32, mybir.dt.float16
AF = mybir.ActivationFunctionType
ALU = mybir.AluOpType

B, S, D, H, HD, FF = 4, 1024, 1024, 16, 64, 4096
NQ = 512            # queries per core
NPAIR = 8           # head pairs
NKT = 8             # k subtiles (128 each)
NQT = 4             # query tiles of 128
HB = 4              # heads per processing block
NBLK = H // HB
EPS = 1e-5
SCALE = 1.0 / math.sqrt(HD)
QS = SCALE * 0.5    # folded into Q^T so score psum = x = raw*SCALE/2
C0, C1, C2, MARGIN = -0.15751157, 0.86990853, 1.83907847, 0.13
NEWTON = 3
NINV = 1.0 / 1024.0

_PROGRAM_CACHE = {}


_LN_CNT = [0]


def ln_stats(nc, pool, xtile, eps_t):
    """bn_stats/bn_aggr mean+var of a (128, 1024) f32 tile -> mv (128,2), rstd (128,1).
    All tiles unique per call so the Sqrt activation never needs >1 sync wait."""
    u = _LN_CNT[0]
    _LN_CNT[0] += 1
    stats = pool.tile([128, 2, 6], f32, tag=f"bn_st{u}", name=f"bn_st{u}")
    for s_ in range(2):
        nc.vector.bn_stats(out=stats[:, s_, :], in_=xtile[:, s_ * 512:(s_ + 1) * 512])
    mv = pool.tile([128, 2], f32, tag=f"bn_mv{u}", name=f"bn_mv{u}")
    nc.vector.bn_aggr(out=mv[:], in_=stats[:])
    sd = pool.tile([128, 1], f32, tag=f"bn_sd{u}", name=f"bn_sd{u}")
    nc.scalar.activation(out=sd[:], in_=mv[:, 1:2], func=AF.Sqrt, bias=eps_t[:])
    rstd = pool.tile([128, 1], f32, tag=f"bn_rs{u}", name=f"bn_rs{u}")
    nc.vector.reciprocal(out=rstd[:], in_=sd[:])
    return mv, rstd


def build_program():
    nc = bacc.Bacc("TRN2", target_bir_lowering=False)

    xb_d = nc.dram_tensor("xb", (S, D), f32, kind="ExternalInput")
    wq_d = nc.dram_tensor("wq16", (D, D), f16, kind="ExternalInput")
    wk_d = nc.dram_tensor("wk16", (D, D), f16, kind="ExternalInput")
    wv_d = nc.dram_tensor("wv16", (D, D), f16, kind="ExternalInput")
    wo_d = nc.dram_tensor("wo16", (D, D), f16, kind="ExternalInput")
    wup_d = nc.dram_tensor("wup16", (D, FF), f16, kind="ExternalInput")
    wdn_d = nc.dram_tensor("wdn16", (FF, D), f16, kind="ExternalInput")
    bqs_d = nc.dram_tensor("bqs", (D, 1), f32, kind="ExternalInput")
    bk_d = nc.dram_tensor("bk_c", (D, 1), f32, kind="ExternalInput")
    bv_d = nc.dram_tensor("bv_row", (1, D), f32, kind="ExternalInput")
    bup_d = nc.dram_tensor("bup_c", (FF, 1), f32, kind="ExternalInput")
    bod_d = nc.dram_tensor("bod_row", (1, D), f32, kind="ExternalInput")
    gf_d = nc.dram_tensor("gf_row", (1, D), f32, kind="ExternalInput")
    bf_d = nc.dram_tensor("bf_row", (1, D), f32, kind="ExternalInput")
    out_d = nc.dram_tensor("out", (NQ, D), f32, kind="ExternalOutput")
    tau_scr = nc.dram_tensor("tau_scr", (H, NQ), f16, kind="Internal")

    def rep_from_dram(pool, dram, name):
        t = pool.tile([128, D], f32, tag=name)
        src = bass.AP(tensor=dram, offset=0, ap=[[0, 128], [1, D]])
        nc.sync.dma_start(out=t[:], in_=src)
        return t

    with tile.TileContext(nc) as tc:
        with ExitStack() as ctx:

            const = ctx.enter_context(tc.tile_pool(name="const", bufs=1))
            persist = ctx.enter_context(tc.tile_pool(name="persist", bufs=1))
            occ_live = ctx.enter_context(tc.tile_pool(name="occ_live", bufs=1))
            qkv_es = ExitStack()
            qkv_live = qkv_es.enter_context(tc.tile_pool(name="qkv_live", bufs=1))
            ph1_ctx = ExitStack()
            ph1 = ph1_ctx.enter_context(tc.tile_pool(name="ph1", bufs=1))

            ident = const.tile([128, 128], f16, tag="ident", name="ident")
            make_identity(nc, ident[:])
            ones_col = const.tile([128, 1], f16, tag="ones_col", name="ones_col")
            nc.vector.memset(ones_col[:], 1.0)
            one_f32 = const.tile([128, 1], f32, tag="one_f32", name="one_f32")
            nc.vector.memset(one_f32[:], 1.0)
            eps_t = const.tile([128, 1], f32, tag="eps_t", name="eps_t")
            nc.vector.memset(eps_t[:], EPS)

            # bias columns to sbuf
            bqs_sb = const.tile([128, 8], f32, tag="bqs_sb", name="bqs_sb")
            nc.sync.dma_start(out=bqs_sb[:], in_=bass.AP(tensor=bqs_d, offset=0, ap=[[1, 128], [128, 8]]))
            bk_sb = const.tile([128, 8], f32, tag="bk_sb", name="bk_sb")
            nc.sync.dma_start(out=bk_sb[:], in_=bass.AP(tensor=bk_d, offset=0, ap=[[1, 128], [128, 8]]))
            bup_sb = const.tile([128, 32], f32, tag="bup_sb", name="bup_sb")
            nc.sync.dma_start(out=bup_sb[:], in_=bass.AP(tensor=bup_d, offset=0, ap=[[1, 128], [128, 32]]))
            bv_rep = rep_from_dram(ph1, bv_d, "bv_rep")

            # ---------------- Phase 1: load x, LN1, y^T, Q^T/K^T/V ----------
            xt = [ph1.tile([128, S], f32, tag=f"x{i}", name=f"x{i}") for i in range(8)]
            for i in range(8):
                eng = [nc.sync, nc.scalar][i % 2]
                eng.dma_start(out=xt[i][:], in_=xb_d[i * 128:(i + 1) * 128, :])

            y16 = []
            with tc.tile_pool(name="ln1", bufs=2) as ln1p, \
                 tc.tile_pool(name="yp", bufs=1) as yp:
                for i in range(8):
                    mv, rstd = ln_stats(nc, ln1p, xt[i], eps_t)
                    yi = yp.tile([128, S], f16, tag=f"y{i}", name=f"y{i}")
                    nc.vector.tensor_scalar(out=yi[:], in0=xt[i][:],
                                            scalar1=mv[:, 0:1], scalar2=rstd[:, 0:1],
                                            op0=ALU.subtract, op1=ALU.mult)
                    y16.append(yi)

                # y^T via PE transpose
                yT = [ph1.tile([128, S], f16, tag=f"yT{d}", name=f"yT{d}") for d in range(8)]
                with tc.tile_pool(name="trp", bufs=4, space="PSUM") as trp:
                    for i in range(8):
                        for dch in range(8):
                            pt = trp.tile([128, 128], f16, tag="trps", name="trps")
                            nc.tensor.transpose(pt[:], y16[i][:, dch * 128:(dch + 1) * 128], ident[:])
                            nc.vector.tensor_copy(out=yT[dch][:, i * 128:(i + 1) * 128], in_=pt[:])

            # Q^T (queries only, scaled), K^T per pair; V per tok tile
            QT = [qkv_live.tile([128, NQ], f16, tag=f"QT{p}", name=f"QT{p}") for p in range(NPAIR)]
            KT = [qkv_live.tile([128, S], f16, tag=f"KT{p}", name=f"KT{p}") for p in range(NPAIR)]
            Vt = [qkv_live.tile([128, D], f16, tag=f"V{i}", name=f"V{i}") for i in range(8)]
            A0sb = persist.tile([128, NPAIR], f32, tag="A0sb", name="A0sb")
            with tc.tile_pool(name="wstr", bufs=6) as wstr, \
                 tc.tile_pool(name="qkvps", bufs=1, space="PSUM") as qkvps, \
                 tc.tile_pool(name="a0ps", bufs=2, space="PSUM") as a0ps:
                for p in range(NPAIR):
                    ps = qkvps.tile([128, NQ], f32, tag="qps", name="qps")
                    for dch in range(8):
                        wsl = wstr.tile([128, 128], f16, tag="wq_sl", name="wq_sl")
                        nc.sync.dma_start(out=wsl[:], in_=wq_d[dch * 128:(dch + 1) * 128, p * 128:(p + 1) * 128])
                        nc.tensor.matmul(out=ps[:], lhsT=wsl[:], rhs=yT[dch][:, 0:NQ],
                                         start=(dch == 0), stop=(dch == 7))
                    nc.scalar.copy(out=ps[:, 0:1], in_=ps[:, 0:1])
                    nc.scalar.activation(out=QT[p][:], in_=ps[:], func=AF.Identity,
                                         bias=bqs_sb[:, p:p + 1], scale=QS)
                for p in range(NPAIR):
                    ps = qkvps.tile([128, S], f32, tag="kps", name="kps")
                    for dch in range(8):
                        wsl = wstr.tile([128, 128], f16, tag="wk_sl", name="wk_sl")
                        nc.sync.dma_start(out=wsl[:], in_=wk_d[dch * 128:(dch + 1) * 128, p * 128:(p + 1) * 128])
                        for half in range(2):
                            nc.tensor.matmul(out=ps[:, half * 512:(half + 1) * 512],
                                             lhsT=wsl[:], rhs=yT[dch][:, half * 512:(half + 1) * 512],
                                             start=(dch == 0), stop=(dch == 7))
                    nc.scalar.copy(out=ps[:, 0:1], in_=ps[:, 0:1])
                    nc.scalar.activation(out=KT[p][:], in_=ps[:], func=AF.Identity,
                                         bias=bk_sb[:, p:p + 1])
                for i in range(8):
                    ps = qkvps.tile([128, D], f32, tag="vps", name="vps")
                    for dch in range(8):
                        wsl = wstr.tile([128, D], f16, tag="wv_sl", name="wv_sl")
                        nc.sync.dma_start(out=wsl[:], in_=wv_d[dch * 128:(dch + 1) * 128, :])
                        for half in range(2):
                            nc.tensor.matmul(out=ps[:, half * 512:(half + 1) * 512],
                                             lhsT=yT[dch][:, i * 128:(i + 1) * 128],
                                             rhs=wsl[:, half * 512:(half + 1) * 512],
                                             start=(dch == 0), stop=(dch == 7))
                    nc.vector.tensor_tensor(out=Vt[i][:], in0=ps[:], in1=bv_rep[:], op=ALU.add)
                # A0 per pair
                for p in range(NPAIR):
                    ps0 = a0ps.tile([128, 1], f32, tag="a0p", name="a0p")
                    for i in range(8):
                        nc.tensor.matmul(out=ps0[:], lhsT=Vt[i][:, p * 128:(p + 1) * 128],
                                         rhs=ones_col[:], start=(i == 0), stop=(i == 7))
                    nc.vector.tensor_copy(out=A0sb[:, p:p + 1], in_=ps0[:])

            # ---------------- Phase 2: attention blocks ----------------------
            ph1_ctx.close()
            occ = [occ_live.tile([128, NQ], f16, tag=f"occ{p}", name=f"occ{p}") for p in range(NPAIR)]
            ph2_ctx = ExitStack()
            ph2 = ph2_ctx.enter_context(tc.tile_pool(name="ph2", bufs=1))

            attn_ctx = ExitStack()
            zpool = attn_ctx.enter_context(tc.tile_pool(name="zpool", bufs=40))
            spool = attn_ctx.enter_context(tc.tile_pool(name="spool", bufs=3))
            stpool = attn_ctx.enter_context(tc.tile_pool(name="stpool", bufs=2))
            reppool = attn_ctx.enter_context(tc.tile_pool(name="reppool", bufs=2))
            rowpool = attn_ctx.enter_context(tc.tile_pool(name="rowpool", bufs=8))
            psA = attn_ctx.enter_context(tc.tile_pool(name="psA", bufs=2, space="PSUM"))
            psT = attn_ctx.enter_context(tc.tile_pool(name="psT", bufs=2, space="PSUM"))
            psCh = attn_ctx.enter_context(tc.tile_pool(name="psCh", bufs=1, space="PSUM"))

            def emit_A(blk):
                heads = list(range(blk * HB, (blk + 1) * HB))
                NC_ = HB * NQT   # stat columns in this block
                NA = 8           # columns on the ACT iteration chain; rest on DVE
                stMx = stpool.tile([128, NC_], f32, tag="stMx", name="stMx")
                stG = stpool.tile([128, NC_], f32, tag="stG", name="stG")
                stH = stpool.tile([128, NC_], f32, tag="stH", name="stH")
                stMV = stpool.tile([128, NC_, 2], f32, tag="stMV", name="stMV")
                tau = stpool.tile([128, NC_], f32, tag="tau", name="tau")
                negtau = stpool.tile([128, NC_], f32, tag="negtau", name="negtau")
                zt = {}
                # scores (A layout) + z + per-col init stats
                for hi, h in enumerate(heads):
                    p, half = h // 2, h % 2
                    hs = slice(half * 64, half * 64 + 64)
                    for t in range(NQT):
                        col = hi * NQT + t
                        ps = psA.tile([128, S], f32, tag="psA", name="psA")
                        for kk in range(2):
                            nc.tensor.matmul(out=ps[:, kk * 512:(kk + 1) * 512],
                                             lhsT=QT[p][hs, t * 128:(t + 1) * 128],
                                             rhs=KT[p][hs, kk * 512:(kk + 1) * 512],
                                             start=True, stop=True)
                        nc.vector.reduce_max(out=stMx[:, col:col + 1], in_=ps[:],
                                             axis=mybir.AxisListType.X, negate=True)
                        zz = zpool.tile([128, S], f16, tag="z", name="z")
                        if blk == 0:
                            nc.vector.memset(zz[:], 0.0)
                        nc.scalar.copy(out=ps[:, 0:1], in_=ps[:, 0:1])
                        nc.scalar.activation(out=zz[:], in_=ps[:], func=AF.Identity,
                                             bias=stMx[:, col:col + 1])
                        zt[col] = zz
                        # init stats: Sz/Sz2 via ACT accum (cols < NA) or bn_stats (DVE)
                        if col < NA:
                            wa = spool.tile([128, S], f16, tag="wa", name="wa")
                            nc.scalar.activation(out=wa[:], in_=zz[:], func=AF.Identity,
                                                 accum_out=stG[:, col:col + 1])
                            wa2 = spool.tile([128, S], f16, tag="wa2", name="wa2")
                            nc.scalar.activation(out=wa2[:], in_=zz[:], func=AF.Square,
                                                 accum_out=stH[:, col:col + 1])
                        else:
                            sb = spool.tile([128, 2, 6], f32, tag="bnw", name="bnw")
                            for s_ in range(2):
                                nc.vector.bn_stats(out=sb[:, s_, :], in_=zz[:, s_ * 512:(s_ + 1) * 512])
                            nc.vector.bn_aggr(out=stMV[:, col, :], in_=sb[:])
                # assemble mean/var batch
                mz = stpool.tile([128, NC_], f32, tag="mz", name="mz")
                varz = stpool.tile([128, NC_], f32, tag="varz", name="varz")
                nc.vector.tensor_scalar(out=mz[:, 0:NA], in0=stG[:, 0:NA], scalar1=NINV, scalar2=None, op0=ALU.mult)
                nc.vector.tensor_scalar(out=varz[:, 0:NA], in0=stH[:, 0:NA], scalar1=NINV, scalar2=None, op0=ALU.mult)
                mzsq = stpool.tile([128, NC_], f32, tag="mzsq", name="mzsq")
                nc.vector.tensor_tensor(out=mzsq[:, 0:NA], in0=mz[:, 0:NA], in1=mz[:, 0:NA], op=ALU.mult)
                nc.vector.tensor_tensor(out=varz[:, 0:NA], in0=varz[:, 0:NA], in1=mzsq[:, 0:NA], op=ALU.subtract)
                nc.vector.tensor_copy(out=mz[:, NA:NC_], in_=stMV[:, NA:NC_, 0])
                nc.vector.tensor_copy(out=varz[:, NA:NC_], in_=stMV[:, NA:NC_, 1])
                nc.vector.tensor_scalar(out=varz[:], in0=varz[:], scalar1=0.0, scalar2=None, op0=ALU.max)
                sdz = stpool.tile([128, NC_], f32, tag="sdz", name="sdz")
                nc.scalar.activation(out=sdz[:], in_=varz[:], func=AF.Sqrt)
                t1_ = stpool.tile([128, NC_], f32, tag="t1_", name="t1_")
                nc.vector.tensor_scalar(out=t1_[:], in0=mz[:], scalar1=C1, scalar2=C0 - MARGIN,
                                        op0=ALU.mult, op1=ALU.add)
                t2_ = stpool.tile([128, NC_], f32, tag="t2_", name="t2_")
                nc.vector.tensor_scalar(out=t2_[:], in0=sdz[:], scalar1=C2, scalar2=None, op0=ALU.mult)
                nc.vector.tensor_tensor(out=tau[:], in0=t1_[:], in1=t2_[:], op=ALU.add)
                nc.vector.tensor_scalar(out=tau[:], in0=tau[:], scalar1=-1e-4, scalar2=None, op0=ALU.min)

                # Newton iterations: ACT chain (Relu+Square accums) / DVE chain (max + bn_stats)
                for it in range(NEWTON):
                    nc.vector.tensor_scalar(out=negtau[:], in0=tau[:], scalar1=-1.0, scalar2=None, op0=ALU.mult)
                    for col in range(NC_):
                        if col < NA:
                            wa = spool.tile([128, S], f16, tag="wa", name="wa")
                            nc.scalar.activation(out=wa[:], in_=zt[col][:], func=AF.Relu,
                                                 bias=negtau[:, col:col + 1],
                                                 accum_out=stG[:, col:col + 1])
                            wa2 = spool.tile([128, S], f16, tag="wa2", name="wa2")
                            nc.scalar.activation(out=wa2[:], in_=wa[:], func=AF.Square,
                                                 accum_out=stH[:, col:col + 1])
                        else:
                            wd = spool.tile([128, S], f16, tag="wd", name="wd")
                            nc.vector.tensor_scalar(out=wd[:], in0=zt[col][:],
                                                    scalar1=tau[:, col:col + 1], scalar2=None,
                                                    op0=ALU.max)
                            sb = spool.tile([128, 2, 6], f32, tag="bnw", name="bnw")
                            for s_ in range(2):
                                nc.vector.bn_stats(out=sb[:, s_, :], in_=wd[:, s_ * 512:(s_ + 1) * 512])
                            nc.vector.bn_aggr(out=stMV[:, col, :], in_=sb[:])
                    # transform DVE cols: A = n*m; SW2 = n*(v+m^2); g = A - n*tau; h = SW2 - 2*tau*A + n*tau^2
                    slc = slice(NA, NC_)
                    a_ = stpool.tile([128, NC_], f32, tag="a_", name="a_")
                    nc.vector.tensor_scalar(out=a_[:, slc], in0=stMV[:, slc, 0], scalar1=1024.0, scalar2=None, op0=ALU.mult)
                    m2_ = stpool.tile([128, NC_], f32, tag="m2_", name="m2_")
                    nc.vector.tensor_tensor(out=m2_[:, slc], in0=stMV[:, slc, 0], in1=stMV[:, slc, 0], op=ALU.mult)
                    sw2 = stpool.tile([128, NC_], f32, tag="sw2", name="sw2")
                    nc.vector.tensor_tensor(out=sw2[:, slc], in0=stMV[:, slc, 1], in1=m2_[:, slc], op=ALU.add)
                    nc.vector.tensor_scalar(out=sw2[:, slc], in0=sw2[:, slc], scalar1=1024.0, scalar2=None, op0=ALU.mult)
                    tg = stpool.tile([128, NC_], f32, tag="tg", name="tg")
                    nc.vector.tensor_scalar(out=tg[:, slc], in0=tau[:, slc], scalar1=-1024.0, scalar2=None, op0=ALU.mult)
                    nc.vector.tensor_tensor(out=stG[:, slc], in0=a_[:, slc], in1=tg[:, slc], op=ALU.add)
                    q1 = stpool.tile([128, NC_], f32, tag="q1", name="q1")
                    nc.vector.tensor_tensor(out=q1[:, slc], in0=tau[:, slc], in1=a_[:, slc], op=ALU.mult)
                    nc.vector.tensor_scalar(out=q1[:, slc], in0=q1[:, slc], scalar1=-2.0, scalar2=None, op0=ALU.mult)
                    tau2 = stpool.tile([128, NC_], f32, tag="tau2", name="tau2")
                    nc.vector.tensor_tensor(out=tau2[:, slc], in0=tau[:, slc], in1=tau[:, slc], op=ALU.mult)
                    nc.vector.tensor_scalar(out=tau2[:, slc], in0=tau2[:, slc], scalar1=1024.0, scalar2=None, op0=ALU.mult)
                    nc.vector.tensor_tensor(out=stH[:, slc], in0=sw2[:, slc], in1=q1[:, slc], op=ALU.add)
                    nc.vector.tensor_tensor(out=stH[:, slc], in0=stH[:, slc], in1=tau2[:, slc], op=ALU.add)
                    # batched Newton update
                    g_ = stpool.tile([128, NC_], f32, tag="g_", name="g_")
                    nc.vector.tensor_scalar(out=g_[:], in0=stG[:], scalar1=1e-6, scalar2=None, op0=ALU.max)
                    rg = stpool.tile([128, NC_], f32, tag="rg", name="rg")
                    nc.vector.reciprocal(out=rg[:], in_=g_[:])
                    h_ = stpool.tile([128, NC_], f32, tag="h_", name="h_")
                    nc.vector.tensor_scalar(out=h_[:], in0=stH[:], scalar1=0.5, scalar2=-0.5,
                                            op0=ALU.mult, op1=ALU.add)
                    dlt = stpool.tile([128, NC_], f32, tag="dlt", name="dlt")
                    nc.vector.tensor_tensor(out=dlt[:], in0=h_[:], in1=rg[:], op=ALU.mult)
                    nc.vector.tensor_tensor(out=tau[:], in0=tau[:], in1=dlt[:], op=ALU.add)
                    nc.vector.tensor_scalar(out=tau[:], in0=tau[:], scalar1=-1e-6, scalar2=None, op0=ALU.min)
                # taux = tau - nmx (x frame); cast f16; rows; replicated tiles
                taux = stpool.tile([128, NC_], f32, tag="taux", name="taux")
                nc.vector.tensor_tensor(out=taux[:], in0=tau[:], in1=stMx[:], op=ALU.subtract)
                tauxf = stpool.tile([128, NC_], f16, tag="tauxf", name="tauxf")
                nc.vector.tensor_copy(out=tauxf[:], in_=taux[:])
                for hi, h in enumerate(heads):
                    for t in range(NQT):
                        col = hi * NQT + t
                        nc.gpsimd.dma_start(out=tau_scr[h:h + 1, t * 128:(t + 1) * 128],
                                          in_=tauxf[:, col:col + 1])
                # per-head full replication for u pass (stride-0 partition DMA from DRAM)
                taurepH = {}
                for h in heads:
                    rp = reppool.tile([128, NQ], f16, tag="taurepH", name="taurepH", bufs=10)
                    nc.gpsimd.dma_start(out=rp[:], in_=bass.AP(
                        tensor=tau_scr, offset=h * NQ, ap=[[0, 128], [1, NQ]]))
                    taurepH[h] = rp
                return taurepH

            def emit_T(blk, taurepH):
                for pp in range(blk * 2, blk * 2 + 2):
                    h0, h1 = 2 * pp, 2 * pp + 1
                    chA1 = psCh.tile([128, NQ], f32, tag="chA1", name="chA1")
                    chA2 = psCh.tile([128, NQ], f32, tag="chA2", name="chA2")
                    for hh_, h in enumerate((h0, h1)):
                        hs = slice(hh_ * 64, hh_ * 64 + 64)
                        for s_ in range(NKT):
                            pst = psT.tile([128, NQ], f32, tag="psT", name="psT")
                            nc.tensor.matmul(out=pst[:],
                                             lhsT=KT[pp][hs, s_ * 128:(s_ + 1) * 128],
                                             rhs=QT[pp][hs, :],
                                             start=True, stop=True)
                            u_ = spool.tile([128, NQ], f16, tag="u_", name="u_")
                            nc.vector.tensor_tensor(out=u_[:], in0=pst[:], in1=taurepH[h][:], op=ALU.max)
                            u2_ = spool.tile([128, NQ], f16, tag="u2_", name="u2_")
                            nc.gpsimd.tensor_tensor(out=u2_[:], in0=u_[:], in1=u_[:], op=ALU.mult)
                            nc.tensor.matmul(out=chA1[hs, :], lhsT=Vt[s_][:, h * 64:(h + 1) * 64],
                                             rhs=u_[:], start=(s_ == 0), stop=(s_ == NKT - 1))
                            nc.tensor.matmul(out=chA2[hs, :], lhsT=Vt[s_][:, h * 64:(h + 1) * 64],
                                             rhs=u2_[:], start=(s_ == 0), stop=(s_ == NKT - 1))
                    # combine: occ = A2 - 2*tau*A1 + tau^2*A0
                    trepP = reppool.tile([128, NQ], f16, tag="trepP", name="trepP")
                    nc.gpsimd.dma_start(out=trepP[0:64, :], in_=bass.AP(
                        tensor=tau_scr, offset=h0 * NQ, ap=[[0, 64], [1, NQ]]))
                    nc.gpsimd.dma_start(out=trepP[64:128, :], in_=bass.AP(
                        tensor=tau_scr, offset=h1 * NQ, ap=[[0, 64], [1, NQ]]))
                    trep2 = reppool.tile([128, NQ], f16, tag="trep2", name="trep2")
                    nc.vector.tensor_scalar(out=trep2[:], in0=trepP[:], scalar1=2.0, scalar2=None, op0=ALU.mult)
                    trepsq = reppool.tile([128, NQ], f16, tag="trepsq", name="trepsq")
                    nc.vector.tensor_tensor(out=trepsq[:], in0=trepP[:], in1=trepP[:], op=ALU.mult)
                    s1_ = spool.tile([128, NQ], f16, tag="s1_", name="s1_")
                    nc.vector.tensor_tensor(out=s1_[:], in0=chA1[:], in1=trep2[:], op=ALU.mult)
                    o1_ = spool.tile([128, NQ], f16, tag="o1_", name="o1_")
                    nc.vector.tensor_tensor(out=o1_[:], in0=chA2[:], in1=s1_[:], op=ALU.subtract)
                    s3_ = spool.tile([128, NQ], f16, tag="s3_", name="s3_")
                    nc.vector.tensor_scalar(out=s3_[:], in0=trepsq[:], scalar1=A0sb[:, pp:pp + 1],
                                            scalar2=None, op0=ALU.mult)
                    nc.vector.tensor_tensor(out=occ[pp][:], in0=o1_[:], in1=s3_[:], op=ALU.add)

            prev = None
            for blk in range(NBLK):
                trh = emit_A(blk)
                if prev is not None:
                    emit_T(prev[0], prev[1])
                prev = (blk, trh)
            emit_T(prev[0], prev[1])

            # ---------------- Phase 3: out-proj + residual + LN2 -------------
            attn_ctx.close()
            ph2_ctx.close()
            qkv_es.close()
            x2_es = ExitStack()
            x2_live = x2_es.enter_context(tc.tile_pool(name="x2_live", bufs=1))
            x2 = [x2_live.tile([128, D], f32, tag=f"x2_{c}", name=f"x2_{c}") for c in range(NQT)]
            ln2_mv = []
            xr = [x2_live.tile([128, D], f32, tag=f"xr{c}", name=f"xr{c}") for c in range(NQT)]
            for c in range(NQT):
                nc.sync.dma_start(out=xr[c][:], in_=xb_d[c * 128:(c + 1) * 128, :])
            with tc.tile_pool(name="wostr", bufs=1) as wostr, \
                 tc.tile_pool(name="pso", bufs=2, space="PSUM") as pso, \
                 tc.tile_pool(name="ln2p", bufs=2) as ln2p:
                wo_sb = [wostr.tile([128, D], f16, tag=f"wo{p}", name=f"wo{p}") for p in range(NPAIR)]
                for p in range(NPAIR):
                    nc.sync.dma_start(out=wo_sb[p][:], in_=wo_d[p * 128:(p + 1) * 128, :])
                for c in range(NQT):
                    ps = pso.tile([128, D], f32, tag="pso", name="pso")
                    for p in range(NPAIR):
                        for half in range(2):
                            nc.tensor.matmul(out=ps[:, half * 512:(half + 1) * 512],
                                             lhsT=occ[p][:, c * 128:(c + 1) * 128],
                                             rhs=wo_sb[p][:, half * 512:(half + 1) * 512],
                                             start=(p == 0), stop=(p == NPAIR - 1))
                    nc.vector.tensor_tensor(out=x2[c][:], in0=ps[:], in1=xr[c][:], op=ALU.add)
                    mv2c, rstd2c = ln_stats(nc, ln2p, x2[c], eps_t)
                    ln2_mv.append((mv2c, rstd2c))

            # LN2 normalize + transpose (stats computed per tile above)
            y2T = [x2_live.tile([128, NQ], f16, tag=f"y2T{d}", name=f"y2T{d}") for d in range(8)]
            with tc.tile_pool(name="y2p", bufs=2) as y2p, \
                 tc.tile_pool(name="tr2ps", bufs=4, space="PSUM") as tr2ps:
                for c in range(NQT):
                    y2c = y2p.tile([128, D], f16, tag="y2c", name="y2c")
                    nc.vector.tensor_scalar(out=y2c[:], in0=x2[c][:],
                                            scalar1=ln2_mv[c][0][:, 0:1], scalar2=ln2_mv[c][1][:, 0:1],
                                            op0=ALU.subtract, op1=ALU.mult)
                    for dch in range(8):
                        pt = tr2ps.tile([128, 128], f16, tag="tr2", name="tr2")
                        nc.tensor.transpose(pt[:], y2c[:, dch * 128:(dch + 1) * 128], ident[:])
                        nc.vector.tensor_copy(out=y2T[dch][:, c * 128:(c + 1) * 128], in_=pt[:])

            # ---------------- Phase 4: FFN ----------------------------------
            ph4_ctx = ExitStack()
            ph4 = ph4_ctx.enter_context(tc.tile_pool(name="ph4", bufs=1))
            bod_rep = rep_from_dram(ph4, bod_d, "bod_rep")
            gf_rep = rep_from_dram(ph4, gf_d, "gf_rep")
            bf_rep = rep_from_dram(ph4, bf_d, "bf_rep")
            hm = [ph4.tile([128, NQ], f16, tag=f"hm{f}", name=f"hm{f}") for f in range(32)]
            with tc.tile_pool(name="ffp", bufs=3) as ffp, \
                 tc.tile_pool(name="wupstr", bufs=10) as wupstr, \
                 tc.tile_pool(name="sppool", bufs=9) as sppool, \
                 tc.tile_pool(name="psu", bufs=2, space="PSUM") as psu:
                GRP = 8
                for g0 in range(0, 32, GRP):
                    spts = {}
                    xbts = {}
                    wg = {}
                    for dch in range(8):
                        wt = wupstr.tile([128, GRP * 128], f16, tag="wup_sl", name="wup_sl", bufs=10)
                        nc.gpsimd.dma_start(out=wt[:], in_=wup_d[dch * 128:(dch + 1) * 128, g0 * 128:(g0 + GRP) * 128])
                        wg[dch] = wt
                    uexs = {}
                    for ff in range(g0, g0 + GRP):
                        ps = psu.tile([128, NQ], f32, tag="psu", name="psu", bufs=4)
                        for dch in range(8):
                            nc.tensor.matmul(out=ps[:], lhsT=wg[dch][:, (ff - g0) * 128:(ff - g0 + 1) * 128],
                                             rhs=y2T[dch][:], start=(dch == 0), stop=(dch == 7))
                        uex = ffp.tile([128, NQ], f32, tag="uex", name="uex", bufs=10)
                        if g0 == 0:
                            nc.vector.memset(uex[:], 0.0)
                        nc.scalar.copy(out=ps[:, 0:1], in_=ps[:, 0:1])
                        nc.scalar.activation(out=uex[:], in_=ps[:], func=AF.Exp,
                                             bias=bup_sb[:, ff:ff + 1])
                        uexs[ff] = uex
                        xbt = sppool.tile([128, NQ], f16, tag="xbt", name="xbt")
                        nc.vector.tensor_scalar(out=xbt[:], in0=ps[:], scalar1=bup_sb[:, ff:ff + 1],
                                                scalar2=None, op0=ALU.add)
                        xbts[ff] = xbt
                    for ff in range(g0, g0 + GRP):
                        spt = sppool.tile([128, NQ], f16, tag="spt", name="spt")
                        if g0 == 0:
                            nc.vector.memset(spt[:], 0.0)
                        nc.scalar.activation(out=spt[:], in_=uexs[ff][:], func=AF.Ln, bias=one_f32[:])
                        spts[ff] = spt
                    for ff in range(g0, g0 + GRP):
                        nc.scalar.activation(out=spts[ff][:], in_=spts[ff][:], func=AF.Tanh)
                        nc.vector.tensor_tensor(out=hm[ff][:], in0=xbts[ff][:], in1=spts[ff][:], op=ALU.mult)

            # down proj + residual + LNf + out
            x3 = [ph4.tile([128, D], f32, tag=f"x3_{c}", name=f"x3_{c}") for c in range(NQT)]
            ln3_mv = {}
            with tc.tile_pool(name="wdstr", bufs=6) as wdstr, \
                 tc.tile_pool(name="psd", bufs=2, space="PSUM") as psd, \
                 tc.tile_pool(name="lnfp", bufs=2) as lnfp:
                for cpair in range(2):
                    cs = [cpair * 2, cpair * 2 + 1]
                    pss = {}
                    for c in cs:
                        pss[c] = psd.tile([128, D], f32, tag=f"psd{c % 2}", name=f"psd{c % 2}")
                    for ff in range(32):
                        wdt = wdstr.tile([128, D], f16, tag="wdt", name="wdt")
                        nc.gpsimd.dma_start(out=wdt[:], in_=wdn_d[ff * 128:(ff + 1) * 128, :])
                        for c in cs:
                            for half in range(2):
                                nc.tensor.matmul(out=pss[c][:, half * 512:(half + 1) * 512],
                                                 lhsT=hm[ff][:, c * 128:(c + 1) * 128],
                                                 rhs=wdt[:, half * 512:(half + 1) * 512],
                                                 start=(ff == 0), stop=(ff == 31))
                    for c in cs:
                        nc.vector.tensor_tensor(out=x3[c][:], in0=pss[c][:], in1=x2[c][:], op=ALU.add)
                        nc.vector.tensor_tensor(out=x3[c][:], in0=x3[c][:], in1=bod_rep[:], op=ALU.add)
                        ln3_mv[c] = ln_stats(nc, lnfp, x3[c], eps_t)

                for c in range(NQT):
                    on = lnfp.tile([128, D], f32, tag="on", name="on")
                    nc.vector.tensor_scalar(out=on[:], in0=x3[c][:],
                                            scalar1=ln3_mv[c][0][:, 0:1], scalar2=ln3_mv[c][1][:, 0:1],
                                            op0=ALU.subtract, op1=ALU.mult)
                    nc.vector.tensor_tensor(out=on[:], in0=on[:], in1=gf_rep[:], op=ALU.mult)
                    nc.vector.tensor_tensor(out=on[:], in0=on[:], in1=bf_rep[:], op=ALU.add)
                    nc.sync.dma_start(out=out_d[c * 128:(c + 1) * 128, :], in_=on[:])
            ph4_ctx.close()
            x2_es.close()

    nc.finalize()
    return nc


def _prep_host(inputs):
    """Fold LN gains/biases into weights; fp16 casts. Returns dict of shared arrays."""
    gi = {k: np.asarray(v) for k, v in inputs.items()}
    f = np.float32
    g1 = gi['ln1_g'].astype(f); b1 = gi['ln1_b'].astype(f)
    g2 = gi['ln2_g'].astype(f); b2 = gi['ln2_b'].astype(f)
    wq = gi['wq'].astype(f); wk = gi['wk'].astype(f); wv = gi['wv'].astype(f)
    shared = {
        'wq16': (wq * g1[:, None]).astype(np.float16),
        'wk16': (wk * g1[:, None]).astype(np.float16),
        'wv16': (wv * g1[:, None]).astype(np.float16),
        'wo16': gi['wo'].astype(f).astype(np.float16),
        'wup16': (gi['w_up'].astype(f) * g2[:, None]).astype(np.float16),
        'wdn16': gi['w_down'].astype(f).astype(np.float16),
        'bqs': ((b1 @ wq + gi['bq'].astype(f)) * QS).reshape(D, 1).astype(f),
        'bk_c': (b1 @ wk + gi['bk'].astype(f)).reshape(D, 1).astype(f),
        'bv_row': (b1 @ wv + gi['bv'].astype(f)).reshape(1, D).astype(f),
        'bup_c': (b2 @ gi['w_up'].astype(f) + gi['b_up'].astype(f)).reshape(FF, 1).astype(f),
        'bod_row': (gi['bo'].astype(f) + gi['b_down'].astype(f)).reshape(1, D).astype(f),
        'gf_row': gi['lnf_g'].astype(f).reshape(1, D),
        'bf_row': gi['lnf_b'].astype(f).reshape(1, D),
    }
    return gi, shared


def make_in_maps(inputs):
    gi, shared = _prep_host(inputs)
    x = gi['x'].astype(np.float32)
    in_maps = []
    for c in range(8):
        b, qh = c // 2, c % 2
        xb = np.roll(x[b], -qh * NQ, axis=0).copy()
        m = {'xb': xb}
        m.update(shared)
        in_maps.append(m)
    return in_maps


def kernel(**inputs):
    from concourse import bass_utils
    key = 'prog'
    if key not in _PROGRAM_CACHE:
        _PROGRAM_CACHE[key] = build_program()
    nc = _PROGRAM_CACHE[key]
    in_maps = make_in_maps(inputs)
    res = bass_utils.run_bass_kernel_spmd(nc, in_maps, core_ids=list(range(8)))
    out = np.zeros((B, S, D), np.float32)
    for c in range(8):
        b, qh = c // 2, c % 2
        out[b, qh * NQ:(qh + 1) * NQ, :] = res.results[c]['out']
    return out


if __name__ == '__main__':
    print("building program...")
    nc = build_program()
    print("built ok; instructions:", len(nc.inst_map))



# revision 17
# speedup vs baseline: 1.2774x; 1.2774x over previous
"""Trainium2 Bass kernel for nn_Encoder_3075196584282 (sparse 1.5-entmax attention encoder).

Self-contained: kernel(**inputs) takes full f32 inputs, shards across 8 NeuronCores
(data-parallel: core = batch*2 + query_half; K/V computed per-core from its batch),
runs one SPMD Bass program, returns the full (4,1024,1024) f32 output.

Entmax-1.5 threshold: tau0 = regression on (max, mean, std) of raw scores, one
Newton step on h(tau)=sum(relu(z-tau))^2=1, then the attention weights are
renormalized per query by S = sum_k p (computed free via a ones-column appended
to V), which absorbs the residual tau error.  The transposed-score pass folds
-tau into the QK^T matmul as a 65th contraction row (ones row in K^T, -tau row
in Q^T), so p = relu(z-tau)^2 needs only one ACT relu + one Pool square and a
single matmul accumulation chain o|S = p @ [V|1].
"""
import math
import numpy as np
from contextlib import ExitStack

import concourse.bass as bass
import concourse.bacc as bacc
import concourse.tile as tile
from concourse import mybir, bass_isa, library_config
from concourse.masks import make_identity

f32, f16 = mybir.dt.float32, mybir.dt.float16
AF = mybir.ActivationFunctionType
ALU = mybir.AluOpType

B, S, D, H, HD, FF = 4, 1024, 1024, 16, 64, 4096
NQ = 512            # queries per core
NPAIR = 8           # head pairs
NKT = 8             # k subtiles (128 each)
NQT = 4             # query tiles of 128
HB = 4              # heads per processing block
NBLK = H // HB
EPS = 1e-5
SCALE = 1.0 / math.sqrt(HD)
QS = SCALE * 0.5    # folded into Q^T so score psum = x = raw*SCALE/2
C0, C1, C2 = -0.15751157, 0.86990853, 1.83907847
NA = 8              # stat columns per block on the ACT chain (rest on DVE)
NINV = 1.0 / 1024.0

_PROGRAM_CACHE = {}

_LN_CNT = [0]


def ln_stats(nc, pool, xtile, eps_t):
    """bn_stats/bn_aggr mean+var of a (128, 1024) f32 tile -> mv (128,2), rstd (128,1)."""
    u = _LN_CNT[0]
    _LN_CNT[0] += 1
    stats = pool.tile([128, 2, 6], f32, tag=f"bn_st{u}", name=f"bn_st{u}")
    for s_ in range(2):
        nc.vector.bn_stats(out=stats[:, s_, :], in_=xtile[:, s_ * 512:(s_ + 1) * 512])
    mv = pool.tile([128, 2], f32, tag=f"bn_mv{u}", name=f"bn_mv{u}")
    nc.vector.bn_aggr(out=mv[:], in_=stats[:])
    sd = pool.tile([128, 1], f32, tag=f"bn_sd{u}", name=f"bn_sd{u}")
    nc.scalar.activation(out=sd[:], in_=mv[:, 1:2], func=AF.Sqrt, bias=eps_t[:])
    rstd = pool.tile([128, 1], f32, tag=f"bn_rs{u}", name=f"bn_rs{u}")
    nc.vector.reciprocal(out=rstd[:], in_=sd[:])
    return mv, rstd


def build_program():
    nc = bacc.Bacc("TRN2", target_bir_lowering=False)

    xb_d = nc.dram_tensor("xb", (S, D), f32, kind="ExternalInput")
    wq_d = nc.dram_tensor("wq16", (D, D), f16, kind="ExternalInput")
    wk_d = nc.dram_tensor("wk16", (D, D), f16, kind="ExternalInput")
    wv_d = nc.dram_tensor("wv16", (D, D), f16, kind="ExternalInput")
    wo_d = nc.dram_tensor("wo16", (D, D), f16, kind="ExternalInput")
    wup_d = nc.dram_tensor("wup16", (D, FF), f16, kind="ExternalInput")
    wdn_d = nc.dram_tensor("wdn16", (FF, D), f16, kind="ExternalInput")
    bqs_d = nc.dram_tensor("bqs", (D, 1), f32, kind="ExternalInput")
    bk_d = nc.dram_tensor("bk_c", (D, 1), f32, kind="ExternalInput")
    bv_d = nc.dram_tensor("bv_row", (1, D), f32, kind="ExternalInput")
    bup_d = nc.dram_tensor("bup_c", (FF, 1), f32, kind="ExternalInput")
    bod_d = nc.dram_tensor("bod_row", (1, D), f32, kind="ExternalInput")
    gf_d = nc.dram_tensor("gf_row", (1, D), f32, kind="ExternalInput")
    bf_d = nc.dram_tensor("bf_row", (1, D), f32, kind="ExternalInput")
    out_d = nc.dram_tensor("out", (NQ, D), f32, kind="ExternalOutput")
    tau_scr = nc.dram_tensor("tau_scr", (H, NQ), f16, kind="Internal")

    def rep_from_dram(pool, dram, name):
        t = pool.tile([128, D], f32, tag=name)
        src = bass.AP(tensor=dram, offset=0, ap=[[0, 128], [1, D]])
        nc.sync.dma_start(out=t[:], in_=src)
        return t

    with tile.TileContext(nc) as tc:
        with ExitStack() as ctx:

            const = ctx.enter_context(tc.tile_pool(name="const", bufs=1))
            occ_live = ctx.enter_context(tc.tile_pool(name="occ_live", bufs=1))
            qkv_es = ExitStack()
            qkv_live = qkv_es.enter_context(tc.tile_pool(name="qkv_live", bufs=1))
            ph1_ctx = ExitStack()
            ph1 = ph1_ctx.enter_context(tc.tile_pool(name="ph1", bufs=1))

            ident = const.tile([128, 128], f16, tag="ident", name="ident")
            make_identity(nc, ident[:])
            eps_t = const.tile([128, 1], f32, tag="eps_t", name="eps_t")
            nc.vector.memset(eps_t[:], EPS)
            one_f32 = const.tile([128, 1], f32, tag="one_f32", name="one_f32")
            nc.vector.memset(one_f32[:], 1.0)

            # bias columns to sbuf
            bqs_sb = const.tile([128, 8], f32, tag="bqs_sb", name="bqs_sb")
            nc.sync.dma_start(out=bqs_sb[:], in_=bass.AP(tensor=bqs_d, offset=0, ap=[[1, 128], [128, 8]]))
            bk_sb = const.tile([128, 8], f32, tag="bk_sb", name="bk_sb")
            nc.sync.dma_start(out=bk_sb[:], in_=bass.AP(tensor=bk_d, offset=0, ap=[[1, 128], [128, 8]]))
            bup_sb = const.tile([128, 32], f32, tag="bup_sb", name="bup_sb")
            nc.sync.dma_start(out=bup_sb[:], in_=bass.AP(tensor=bup_d, offset=0, ap=[[1, 128], [128, 32]]))
            bv_rep = rep_from_dram(ph1, bv_d, "bv_rep")

            # ---------------- Phase 1: load x, LN1, y^T, Q^T/K^T/V ----------
            xt = [ph1.tile([128, S], f32, tag=f"x{i}", name=f"x{i}") for i in range(8)]
            for i in range(8):
                eng = [nc.sync, nc.scalar][i % 2]
                eng.dma_start(out=xt[i][:], in_=xb_d[i * 128:(i + 1) * 128, :])
            # prefetch residual x for phase 3 early (no deps)
            xr = [occ_live.tile([128, D], f32, tag=f"xr{c}", name=f"xr{c}") for c in range(NQT)]
            for c in range(NQT):
                nc.scalar.dma_start(out=xr[c][:], in_=xb_d[c * 128:(c + 1) * 128, :])

            y16 = []
            with tc.tile_pool(name="ln1", bufs=2) as ln1p, \
                 tc.tile_pool(name="yp", bufs=1) as yp:
                for i in range(8):
                    mv, rstd = ln_stats(nc, ln1p, xt[i], eps_t)
                    yi = yp.tile([128, S], f16, tag=f"y{i}", name=f"y{i}")
                    nc.vector.tensor_scalar(out=yi[:], in0=xt[i][:],
                                            scalar1=mv[:, 0:1], scalar2=rstd[:, 0:1],
                                            op0=ALU.subtract, op1=ALU.mult)
                    y16.append(yi)

                # y^T via PE transpose
                yT = [ph1.tile([128, S], f16, tag=f"yT{d}", name=f"yT{d}") for d in range(8)]
                with tc.tile_pool(name="trp", bufs=4, space="PSUM") as trp:
                    for i in range(8):
                        for dch in range(8):
                            pt = trp.tile([128, 128], f16, tag="trps", name="trps")
                            nc.tensor.transpose(pt[:], y16[i][:, dch * 128:(dch + 1) * 128], ident[:])
                            nc.vector.tensor_copy(out=yT[dch][:, i * 128:(i + 1) * 128], in_=pt[:])

            # Q^T per head [65, NQ] (row 64 <- -tau later), K^T per head [65, S]
            # (row 64 = ones), V per tok tile [128, 16, 65] (col 64 of each head = 1)
            # 96-row Q^T/K^T: rows 0-63 head dims, row 64 = ones (K) / -tau (Q),
            # rows 65-95 zero padding (PE row groups are 32-aligned).
            QT = [qkv_live.tile([96, NQ], f16, tag=f"QT{h}", name=f"QT{h}") for h in range(H)]
            KT = [qkv_live.tile([96, S], f16, tag=f"KT{h}", name=f"KT{h}") for h in range(H)]
            Vt = [qkv_live.tile([128, 16, 128], f16, tag=f"V{i}", name=f"V{i}") for i in range(8)]
            for h in range(H):
                nc.vector.memset(KT[h][64:96, :], 0.0)
                nc.vector.memset(KT[h][64:65, :], 1.0)
                nc.vector.memset(QT[h][64:96, :], 0.0)
            for i in range(8):
                nc.vector.memset(Vt[i][:, :, 64:128], 1.0)
            with tc.tile_pool(name="wstr", bufs=6) as wstr, \
                 tc.tile_pool(name="qkvps", bufs=1, space="PSUM") as qkvps:
                for p in range(NPAIR):
                    ps = qkvps.tile([128, NQ], f32, tag="qps", name="qps")
                    for dch in range(8):
                        wsl = wstr.tile([128, 128], f16, tag="wq_sl", name="wq_sl")
                        nc.sync.dma_start(out=wsl[:], in_=wq_d[dch * 128:(dch + 1) * 128, p * 128:(p + 1) * 128])
                        nc.tensor.matmul(out=ps[:], lhsT=wsl[:], rhs=yT[dch][:, 0:NQ],
                                         start=(dch == 0), stop=(dch == 7))
                    nc.scalar.copy(out=ps[:, 0:1], in_=ps[:, 0:1])
                    for hh in range(2):
                        nc.scalar.activation(out=QT[2 * p + hh][0:64, :], in_=ps[hh * 64:(hh + 1) * 64, :],
                                             func=AF.Identity, bias=bqs_sb[hh * 64:(hh + 1) * 64, p:p + 1],
                                             scale=QS)
                for p in range(NPAIR):
                    ps = qkvps.tile([128, S], f32, tag="kps", name="kps")
                    for dch in range(8):
                        wsl = wstr.tile([128, 128], f16, tag="wk_sl", name="wk_sl")
                        nc.sync.dma_start(out=wsl[:], in_=wk_d[dch * 128:(dch + 1) * 128, p * 128:(p + 1) * 128])
                        for half in range(2):
                            nc.tensor.matmul(out=ps[:, half * 512:(half + 1) * 512],
                                             lhsT=wsl[:], rhs=yT[dch][:, half * 512:(half + 1) * 512],
                                             start=(dch == 0), stop=(dch == 7))
                    nc.scalar.copy(out=ps[:, 0:1], in_=ps[:, 0:1])
                    for hh in range(2):
                        nc.scalar.activation(out=KT[2 * p + hh][0:64, :], in_=ps[hh * 64:(hh + 1) * 64, :],
                                             func=AF.Identity, bias=bk_sb[hh * 64:(hh + 1) * 64, p:p + 1])
                for i in range(8):
                    ps = qkvps.tile([128, 16, 64], f32, tag="vps", name="vps")
                    for dch in range(8):
                        wsl = wstr.tile([128, D], f16, tag="wv_sl", name="wv_sl")
                        nc.sync.dma_start(out=wsl[:], in_=wv_d[dch * 128:(dch + 1) * 128, :])
                        for half in range(2):
                            nc.tensor.matmul(out=ps[:, half * 8:(half + 1) * 8, :],
                                             lhsT=yT[dch][:, i * 128:(i + 1) * 128],
                                             rhs=wsl[:, half * 512:(half + 1) * 512],
                                             start=(dch == 0), stop=(dch == 7))
                    nc.vector.tensor_tensor(out=Vt[i][:, :, 0:64], in0=ps[:, :, :],
                                            in1=bv_rep[:, 0:1024], op=ALU.add)

            # ---------------- Phase 2: attention blocks ----------------------
            ph1_ctx.close()
            occ = [occ_live.tile([128, NQ], f16, tag=f"occ{p}", name=f"occ{p}") for p in range(NPAIR)]
            attn_ctx = ExitStack()
            zpool = attn_ctx.enter_context(tc.tile_pool(name="zpool", bufs=24))
            spool = attn_ctx.enter_context(tc.tile_pool(name="spool", bufs=4))
            stpool = attn_ctx.enter_context(tc.tile_pool(name="stpool", bufs=2))
            rppool = attn_ctx.enter_context(tc.tile_pool(name="rppool", bufs=2))
            psA = attn_ctx.enter_context(tc.tile_pool(name="psA", bufs=2, space="PSUM"))
            psT = attn_ctx.enter_context(tc.tile_pool(name="psT", bufs=2, space="PSUM"))
            psCh = attn_ctx.enter_context(tc.tile_pool(name="psCh", bufs=2, space="PSUM"))

            def emit_A(blk):
                heads = list(range(blk * HB, (blk + 1) * HB))
                NC_ = HB * NQT   # stat columns in this block
                stMx = stpool.tile([128, NC_], f32, tag="stMx", name="stMx")
                stG = stpool.tile([128, NC_], f32, tag="stG", name="stG")
                stH = stpool.tile([128, NC_], f32, tag="stH", name="stH")
                tau = stpool.tile([128, NC_], f32, tag="tau", name="tau")
                stMV = stpool.tile([128, NC_, 2], f32, tag="stMV", name="stMV")
                zt = {}
                # scores (A layout) + raw z copy + bn stats + max
                for hi, h in enumerate(heads):
                    for t in range(NQT):
                        col = hi * NQT + t
                        ps = psA.tile([128, S], f32, tag="psA", name="psA")
                        for kk in range(2):
                            nc.tensor.matmul(out=ps[:, kk * 512:(kk + 1) * 512],
                                             lhsT=QT[h][0:64, t * 128:(t + 1) * 128],
                                             rhs=KT[h][0:64, kk * 512:(kk + 1) * 512],
                                             start=True, stop=True)
                        zz = zpool.tile([128, S], f16, tag="z", name="z")
                        nc.scalar.copy(out=ps[:, 0:1], in_=ps[:, 0:1])
                        nc.scalar.activation(out=zz[:], in_=ps[:], func=AF.Identity)
                        zt[col] = zz
                        sb = spool.tile([128, 2, 6], f32, tag="bnw", name="bnw")
                        for s_ in range(2):
                            nc.vector.bn_stats(out=sb[:, s_, :], in_=zz[:, s_ * 512:(s_ + 1) * 512])
                        nc.vector.bn_aggr(out=stMV[:, col, :], in_=sb[:])
                        nc.vector.reduce_max(out=stMx[:, col:col + 1], in_=zz[:],
                                             axis=mybir.AxisListType.X)
                # tau0 = C0 + C1*mz + (1-C1)*mx + C2*std  (raw frame, margin 0)
                varz = stpool.tile([128, NC_], f32, tag="varz", name="varz")
                nc.vector.tensor_scalar(out=varz[:], in0=stMV[:, :, 1], scalar1=0.0, scalar2=None, op0=ALU.max)
                sdz = stpool.tile([128, NC_], f32, tag="sdz", name="sdz")
                nc.scalar.activation(out=sdz[:], in_=varz[:], func=AF.Sqrt)
                mxe = stpool.tile([128, NC_], f32, tag="mxe", name="mxe")
                nc.vector.tensor_scalar(out=mxe[:], in0=stMx[:], scalar1=-1e-6, scalar2=None, op0=ALU.add)
                t1_ = stpool.tile([128, NC_], f32, tag="t1_", name="t1_")
                nc.vector.tensor_scalar(out=t1_[:], in0=stMV[:, :, 0], scalar1=C1, scalar2=C0,
                                        op0=ALU.mult, op1=ALU.add)
                t2_ = stpool.tile([128, NC_], f32, tag="t2_", name="t2_")
                nc.vector.tensor_scalar(out=t2_[:], in0=sdz[:], scalar1=C2, scalar2=None, op0=ALU.mult)
                t3_ = stpool.tile([128, NC_], f32, tag="t3_", name="t3_")
                nc.vector.tensor_scalar(out=t3_[:], in0=stMx[:], scalar1=1.0 - C1, scalar2=None, op0=ALU.mult)
                nc.vector.tensor_tensor(out=tau[:], in0=t1_[:], in1=t2_[:], op=ALU.add)
                nc.vector.tensor_tensor(out=tau[:], in0=tau[:], in1=t3_[:], op=ALU.add)
                nc.vector.tensor_tensor(out=tau[:], in0=tau[:], in1=mxe[:], op=ALU.min)

                # one Newton step: g = sum relu(z-tau); h = sum relu(z-tau)^2
                negtau = stpool.tile([128, NC_], f32, tag="negtau", name="negtau")
                nc.vector.tensor_scalar(out=negtau[:], in0=tau[:], scalar1=-1.0, scalar2=None, op0=ALU.mult)
                for col in range(NC_):
                    if col < NA:
                        wa = spool.tile([128, S], f16, tag="wa", name="wa")
                        nc.scalar.activation(out=wa[:], in_=zt[col][:], func=AF.Relu,
                                             bias=negtau[:, col:col + 1],
                                             accum_out=stG[:, col:col + 1])
                        wa2 = spool.tile([128, S], f16, tag="wa2", name="wa2")
                        nc.scalar.activation(out=wa2[:], in_=wa[:], func=AF.Square,
                                             accum_out=stH[:, col:col + 1])
                    else:
                        wd = spool.tile([128, S], f16, tag="wd", name="wd")
                        nc.vector.tensor_scalar(out=wd[:], in0=zt[col][:],
                                                scalar1=tau[:, col:col + 1], scalar2=None,
                                                op0=ALU.max)
                        sb = spool.tile([128, 2, 6], f32, tag="bnw", name="bnw")
                        for s_ in range(2):
                            nc.vector.bn_stats(out=sb[:, s_, :], in_=wd[:, s_ * 512:(s_ + 1) * 512])
                        nc.vector.bn_aggr(out=stMV[:, col, :], in_=sb[:])
                # transform DVE cols: A = n*m; SW2 = n*(v+m^2); g = A - n*tau;
                # h = SW2 - 2*tau*A + n*tau^2
                slc = slice(NA, NC_)
                a_ = stpool.tile([128, NC_], f32, tag="a_", name="a_")
                nc.vector.tensor_scalar(out=a_[:, slc], in0=stMV[:, slc, 0], scalar1=1024.0, scalar2=None, op0=ALU.mult)
                m2_ = stpool.tile([128, NC_], f32, tag="m2_", name="m2_")
                nc.vector.tensor_tensor(out=m2_[:, slc], in0=stMV[:, slc, 0], in1=stMV[:, slc, 0], op=ALU.mult)
                sw2 = stpool.tile([128, NC_], f32, tag="sw2", name="sw2")
                nc.vector.tensor_tensor(out=sw2[:, slc], in0=stMV[:, slc, 1], in1=m2_[:, slc], op=ALU.add)
                nc.vector.tensor_scalar(out=sw2[:, slc], in0=sw2[:, slc], scalar1=1024.0, scalar2=None, op0=ALU.mult)
                tg = stpool.tile([128, NC_], f32, tag="tg", name="tg")
                nc.vector.tensor_scalar(out=tg[:, slc], in0=tau[:, slc], scalar1=-1024.0, scalar2=None, op0=ALU.mult)
                nc.vector.tensor_tensor(out=stG[:, slc], in0=a_[:, slc], in1=tg[:, slc], op=ALU.add)
                q1 = stpool.tile([128, NC_], f32, tag="q1", name="q1")
                nc.vector.tensor_tensor(out=q1[:, slc], in0=tau[:, slc], in1=a_[:, slc], op=ALU.mult)
                nc.vector.tensor_scalar(out=q1[:, slc], in0=q1[:, slc], scalar1=-2.0, scalar2=None, op0=ALU.mult)
                tau2 = stpool.tile([128, NC_], f32, tag="tau2", name="tau2")
                nc.vector.tensor_tensor(out=tau2[:, slc], in0=tau[:, slc], in1=tau[:, slc], op=ALU.mult)
                nc.vector.tensor_scalar(out=tau2[:, slc], in0=tau2[:, slc], scalar1=1024.0, scalar2=None, op0=ALU.mult)
                nc.vector.tensor_tensor(out=stH[:, slc], in0=sw2[:, slc], in1=q1[:, slc], op=ALU.add)
                nc.vector.tensor_tensor(out=stH[:, slc], in0=stH[:, slc], in1=tau2[:, slc], op=ALU.add)
                # batched Newton update: tau += (0.5h - 0.5)/max(g, 1e-6); clip < mx
                g_ = stpool.tile([128, NC_], f32, tag="g_", name="g_")
                nc.vector.tensor_scalar(out=g_[:], in0=stG[:], scalar1=1e-6, scalar2=None, op0=ALU.max)
                rg = stpool.tile([128, NC_], f32, tag="rg", name="rg")
                nc.vector.reciprocal(out=rg[:], in_=g_[:])
                h_ = stpool.tile([128, NC_], f32, tag="h_", name="h_")
                nc.vector.tensor_scalar(out=h_[:], in0=stH[:], scalar1=0.5, scalar2=-0.5,
                                        op0=ALU.mult, op1=ALU.add)
                dlt = stpool.tile([128, NC_], f32, tag="dlt", name="dlt")
                nc.vector.tensor_tensor(out=dlt[:], in0=h_[:], in1=rg[:], op=ALU.mult)
                nc.vector.tensor_tensor(out=tau[:], in0=tau[:], in1=dlt[:], op=ALU.add)
                nc.vector.tensor_tensor(out=tau[:], in0=tau[:], in1=mxe[:], op=ALU.min)
                # -tau as f16, to DRAM rows, back into QT row 64 per head
                tnegf = stpool.tile([128, NC_], f16, tag="tnegf", name="tnegf")
                nc.vector.tensor_scalar(out=tnegf[:], in0=tau[:], scalar1=-1.0, scalar2=None, op0=ALU.mult)
                for hi, h in enumerate(heads):
                    nc.gpsimd.dma_start(
                        out=bass.AP(tensor=tau_scr, offset=h * NQ, ap=[[1, 128], [128, NQT]]),
                        in_=tnegf[:, hi * NQT:(hi + 1) * NQT])
                for h in heads:
                    nc.gpsimd.dma_start(out=QT[h][64:65, :],
                                        in_=bass.AP(tensor=tau_scr, offset=h * NQ, ap=[[0, 1], [1, NQ]]))

            def emit_T(blk):
                for h in range(blk * HB, (blk + 1) * HB):
                    pp, hh = h // 2, h % 2
                    chR = psCh.tile([128, NQ], f32, tag="chR", name="chR")
                    for s_ in range(NKT):
                        pst = psT.tile([128, NQ], f32, tag="psT", name="psT")
                        nc.tensor.matmul(out=pst[:],
                                         lhsT=KT[h][:, s_ * 128:(s_ + 1) * 128],
                                         rhs=QT[h][:, :],
                                         start=True, stop=True)
                        r_ = spool.tile([128, NQ], f16, tag="r_", name="r_")
                        nc.vector.tensor_scalar(out=r_[:], in0=pst[:], scalar1=0.0,
                                                scalar2=None, op0=ALU.max)
                        p_ = spool.tile([128, NQ], f16, tag="p_", name="p_")
                        nc.gpsimd.tensor_tensor(out=p_[:], in0=r_[:], in1=r_[:], op=ALU.mult)
                        nc.tensor.matmul(out=chR[:], lhsT=Vt[s_][:, h, :],
                                         rhs=p_[:], start=(s_ == 0), stop=(s_ == NKT - 1))
                    # normalize: occ = chR[0:64] / S (rows 64:128 = S replicated)
                    ssb = rppool.tile([64, NQ], f32, tag="ssb", name="ssb")
                    nc.vector.tensor_copy(out=ssb[:], in_=chR[64:128, :])
                    rsrep = rppool.tile([64, NQ], f32, tag="rsrep", name="rsrep")
                    nc.vector.reciprocal_approx_fast(out=rsrep[:], in_=ssb[:])
                    nc.vector.tensor_tensor(out=occ[pp][hh * 64:(hh + 1) * 64, :],
                                            in0=chR[0:64, :], in1=rsrep[:], op=ALU.mult)

            prev = None
            for blk in range(NBLK):
                emit_A(blk)
                if prev is not None:
                    emit_T(prev)
                prev = blk
            emit_T(prev)

            # ---------------- Phase 3: out-proj + residual + LN2 -------------
            attn_ctx.close()
            qkv_es.close()
            x2_es = ExitStack()
            x2_live = x2_es.enter_context(tc.tile_pool(name="x2_live", bufs=1))
            x2 = [x2_live.tile([128, D], f32, tag=f"x2_{c}", name=f"x2_{c}") for c in range(NQT)]
            ln2_mv = []
            with tc.tile_pool(name="wostr", bufs=1) as wostr, \
                 tc.tile_pool(name="pso", bufs=2, space="PSUM") as pso, \
                 tc.tile_pool(name="ln2p", bufs=2) as ln2p:
                wo_sb = [wostr.tile([128, D], f16, tag=f"wo{p}", name=f"wo{p}") for p in range(NPAIR)]
                for p in range(NPAIR):
                    nc.sync.dma_start(out=wo_sb[p][:], in_=wo_d[p * 128:(p + 1) * 128, :])
                for c in range(NQT):
                    ps = pso.tile([128, D], f32, tag="pso", name="pso")
                    for p in range(NPAIR):
                        for half in range(2):
                            nc.tensor.matmul(out=ps[:, half * 512:(half + 1) * 512],
                                             lhsT=occ[p][:, c * 128:(c + 1) * 128],
                                             rhs=wo_sb[p][:, half * 512:(half + 1) * 512],
                                             start=(p == 0), stop=(p == NPAIR - 1))
                    nc.vector.tensor_tensor(out=x2[c][:], in0=ps[:], in1=xr[c][:], op=ALU.add)
                    mv2c, rstd2c = ln_stats(nc, ln2p, x2[c], eps_t)
                    ln2_mv.append((mv2c, rstd2c))

            # LN2 normalize + transpose (stats computed per tile above)
            y2T = [x2_live.tile([128, NQ], f16, tag=f"y2T{d}", name=f"y2T{d}") for d in range(8)]
            with tc.tile_pool(name="y2p", bufs=2) as y2p, \
                 tc.tile_pool(name="tr2ps", bufs=4, space="PSUM") as tr2ps:
                for c in range(NQT):
                    y2c = y2p.tile([128, D], f16, tag="y2c", name="y2c")
                    nc.vector.tensor_scalar(out=y2c[:], in0=x2[c][:],
                                            scalar1=ln2_mv[c][0][:, 0:1], scalar2=ln2_mv[c][1][:, 0:1],
                                            op0=ALU.subtract, op1=ALU.mult)
                    for dch in range(8):
                        pt = tr2ps.tile([128, 128], f16, tag="tr2", name="tr2")
                        nc.tensor.transpose(pt[:], y2c[:, dch * 128:(dch + 1) * 128], ident[:])
                        nc.vector.tensor_copy(out=y2T[dch][:, c * 128:(c + 1) * 128], in_=pt[:])

            # ---------------- Phase 4: FFN (Mish via ACT table) --------------
            ph4_ctx = ExitStack()
            ph4 = ph4_ctx.enter_context(tc.tile_pool(name="ph4", bufs=1))
            bod_rep = rep_from_dram(ph4, bod_d, "bod_rep")
            gf_rep = rep_from_dram(ph4, gf_d, "gf_rep")
            bf_rep = rep_from_dram(ph4, bf_d, "bf_rep")
            hm = [ph4.tile([128, NQ], f16, tag=f"hm{f}", name=f"hm{f}") for f in range(32)]
            sps = [ph4.tile([128, NQ], f16, tag=f"sp{f}", name=f"sp{f}") for f in range(32)]
            xbts = [ph4.tile([128, NQ], f16, tag=f"xbt{f}", name=f"xbt{f}") for f in range(32)]
            with tc.tile_pool(name="wupstr", bufs=10) as wupstr, \
                 tc.tile_pool(name="ffp", bufs=4) as ffp, \
                 tc.tile_pool(name="psu", bufs=4, space="PSUM") as psu:
                GRP = 8
                for g0 in range(0, 32, GRP):
                    wg = {}
                    for dch in range(8):
                        wt = wupstr.tile([128, GRP * 128], f16, tag="wup_sl", name="wup_sl", bufs=10)
                        nc.gpsimd.dma_start(out=wt[:], in_=wup_d[dch * 128:(dch + 1) * 128, g0 * 128:(g0 + GRP) * 128])
                        wg[dch] = wt
                    for ff in range(g0, g0 + GRP):
                        ps = psu.tile([128, NQ], f32, tag="psu", name="psu", bufs=4)
                        for dch in range(8):
                            nc.tensor.matmul(out=ps[:], lhsT=wg[dch][:, (ff - g0) * 128:(ff - g0 + 1) * 128],
                                             rhs=y2T[dch][:], start=(dch == 0), stop=(dch == 7))
                        nc.scalar.copy(out=ps[:, 0:1], in_=ps[:, 0:1])
                        uex = ffp.tile([128, NQ], f32, tag="uex", name="uex")
                        nc.scalar.activation(out=uex[:], in_=ps[:], func=AF.Exp,
                                             bias=bup_sb[:, ff:ff + 1])
                        nc.vector.tensor_scalar(out=xbts[ff][:], in0=ps[:], scalar1=bup_sb[:, ff:ff + 1],
                                                scalar2=None, op0=ALU.add)
                        nc.scalar.activation(out=sps[ff][:], in_=uex[:], func=AF.Ln, bias=one_f32[:])
                # tanh batched last: single activation-table switch
                for ff in range(32):
                    nc.scalar.activation(out=sps[ff][:], in_=sps[ff][:], func=AF.Tanh)
                    nc.vector.tensor_tensor(out=hm[ff][:], in0=xbts[ff][:], in1=sps[ff][:], op=ALU.mult)

            # down proj + residual + LNf + out
            x3 = [ph4.tile([128, D], f32, tag=f"x3_{c}", name=f"x3_{c}") for c in range(NQT)]
            ln3_mv = {}
            with tc.tile_pool(name="wdstr", bufs=6) as wdstr, \
                 tc.tile_pool(name="psd", bufs=2, space="PSUM") as psd, \
                 tc.tile_pool(name="lnfp", bufs=2) as lnfp:
                for cpair in range(2):
                    cs = [cpair * 2, cpair * 2 + 1]
                    pss = {}
                    for c in cs:
                        pss[c] = psd.tile([128, D], f32, tag=f"psd{c % 2}", name=f"psd{c % 2}")
                    for ff in range(32):
                        wdt = wdstr.tile([128, D], f16, tag="wdt", name="wdt")
                        nc.gpsimd.dma_start(out=wdt[:], in_=wdn_d[ff * 128:(ff + 1) * 128, :])
                        for c in cs:
                            for half in range(2):
                                nc.tensor.matmul(out=pss[c][:, half * 512:(half + 1) * 512],
                                                 lhsT=hm[ff][:, c * 128:(c + 1) * 128],
                                                 rhs=wdt[:, half * 512:(half + 1) * 512],
                                                 start=(ff == 0), stop=(ff == 31))
                    for c in cs:
                        nc.vector.tensor_tensor(out=x3[c][:], in0=pss[c][:], in1=x2[c][:], op=ALU.add)
                        nc.vector.tensor_tensor(out=x3[c][:], in0=x3[c][:], in1=bod_rep[:], op=ALU.add)
                        ln3_mv[c] = ln_stats(nc, lnfp, x3[c], eps_t)

                for c in range(NQT):
                    on = lnfp.tile([128, D], f32, tag="on", name="on")
                    nc.vector.tensor_scalar(out=on[:], in0=x3[c][:],
                                            scalar1=ln3_mv[c][0][:, 0:1], scalar2=ln3_mv[c][1][:, 0:1],
                                            op0=ALU.subtract, op1=ALU.mult)
                    nc.vector.tensor_tensor(out=on[:], in0=on[:], in1=gf_rep[:], op=ALU.mult)
                    nc.vector.tensor_tensor(out=on[:], in0=on[:], in1=bf_rep[:], op=ALU.add)
                    nc.sync.dma_start(out=out_d[c * 128:(c + 1) * 128, :], in_=on[:])
            ph4_ctx.close()
            x2_es.close()

    nc.finalize()
    return nc


def _prep_host(inputs):
    """Fold LN gains/biases into weights; fp16 casts. Returns dict of shared arrays."""
    gi = {k: np.asarray(v) for k, v in inputs.items()}
    f = np.float32
    g1 = gi['ln1_g'].astype(f); b1 = gi['ln1_b'].astype(f)
    g2 = gi['ln2_g'].astype(f); b2 = gi['ln2_b'].astype(f)
    wq = gi['wq'].astype(f); wk = gi['wk'].astype(f); wv = gi['wv'].astype(f)
    shared = {
        'wq16': (wq * g1[:, None]).astype(np.float16),
        'wk16': (wk * g1[:, None]).astype(np.float16),
        'wv16': (wv * g1[:, None]).astype(np.float16),
        'wo16': gi['wo'].astype(f).astype(np.float16),
        'wup16': (gi['w_up'].astype(f) * g2[:, None]).astype(np.float16),
        'wdn16': gi['w_down'].astype(f).astype(np.float16),
        'bqs': ((b1 @ wq + gi['bq'].astype(f)) * QS).reshape(D, 1).astype(f),
        'bk_c': (b1 @ wk + gi['bk'].astype(f)).reshape(D, 1).astype(f),
        'bv_row': (b1 @ wv + gi['bv'].astype(f)).reshape(1, D).astype(f),
        'bup_c': (b2 @ gi['w_up'].astype(f) + gi['b_up'].astype(f)).reshape(FF, 1).astype(f),
        'bod_row': (gi['bo'].astype(f) + gi['b_down'].astype(f)).reshape(1, D).astype(f),
        'gf_row': gi['lnf_g'].astype(f).reshape(1, D),
        'bf_row': gi['lnf_b'].astype(f).reshape(1, D),
    }
    return gi, shared


def make_in_maps(inputs):
    gi, shared = _prep_host(inputs)
    x = gi['x'].astype(np.float32)
    in_maps = []
    for c in range(8):
        b, qh = c // 2, c % 2
        xb = np.roll(x[b], -qh * NQ, axis=0).copy()
        m = {'xb': xb}
        m.update(shared)
        in_maps.append(m)
    return in_maps


def kernel(**inputs):
    from concourse import bass_utils
    key = 'prog'
    if key not in _PROGRAM_CACHE:
        _PROGRAM_CACHE[key] = build_program()
    nc = _PROGRAM_CACHE[key]
    in_maps = make_in_maps(inputs)
    res = bass_utils.run_bass_kernel_spmd(nc, in_maps, core_ids=list(range(8)))
    out = np.zeros((B, S, D), np.float32)
    for c in range(8):
        b, qh = c // 2, c % 2
        out[b, qh * NQ:(qh + 1) * NQ, :] = res.results[c]['out']
    return out


if __name__ == '__main__':
    print("building program...")
    nc = build_program()
    print("built ok; instructions:", len(nc.inst_map))
